# revision 2
# baseline (speedup 1.0000x reference)
"""Trainium2 Bass kernel v3: multi-head encoder-decoder attention.

nn_MultiHeadEncDecAttention — B=1, N=4096, d_model=768, 12 heads, d_k=64.

Sharding: core pair p in {0..3} owns heads {3p, 3p+1, 3p+2}; within a
pair, core 2p handles query rows [0, 2048) and core 2p+1 rows [2048, 4096).

v3 design vs v2 (v2 = fp16 software-pipelined, ACT-bound at ~200us/core
of softmax exp):
  - exp split across ACT (exact spline) and DVE (Schraudolph int16 bit
    trick: i16 = round(s_raw*(0.125*1024/ln2) + 15315), int16 bits
    reinterpreted as fp16 give exp(s/8)*(1 +- 3% sawtooth)). 6 of 16
    chunk positions run on DVE; softmax-ratio cancellation keeps total
    rel err at ~7.8e-3 vs the 2e-2 gate. This breaks the ACT wall.
  - AV matmuls q-split col-tiled: A (q 0:255) -> PSUM parts 0:64 at col
    position (0,0), B (q 256:511) -> parts 64:128 at (0,64), same V(kt)
    stationary. The two FD=256 matmuls run concurrently (distinct PE
    col-groups) -> ~2x AV throughput vs v2's 64-wide stationary padded
    to 128 with a ones column.
  - Denominator via a separate 4-way col-tiled pass: all-ones [128,32]
    stationaries at col positions 0/32/64/96 accumulate quarter-strip
    partials (strips = (even/odd kt) x (q-lo/q-hi)), folded by a
    [128,2]-of-1/32 stationary matmul, then DVE recip + a [2,128]
    broadcast matmul.
  - o tiles ([128,256]: parts 0:64 q-lo, 64:128 q-hi) are relocated to
    the baseline [64,512] layout with two SBUF->SBUF DMAs so outproj
    stays all-(0,0). (Row-positioned (64,0) matmuls interleaved with
    the col-tiled AV/den stream wedge the exec unit -
    NRT_EXEC_UNIT_UNRECOVERABLE - as do col-32 tiles at non-zero
    within-bank PSUM offsets; every sub-128-wide matmul here therefore
    writes at within-bank offset 0 and outproj avoids row positions.)
  - everything fp16 (no fp8): AV numerics exact, total rel err ~7.8e-3.
"""

import sys

sys.path.insert(0, "/opt/trn_rl_repo")

from contextlib import ExitStack

import numpy as np

import concourse.tile as tile
from concourse import bacc, mybir
from concourse.bass_utils import run_bass_kernel_spmd

F32 = mybir.dt.float32
BF16 = mybir.dt.float16   # fp16: PE rate = bf16, 3 extra mantissa bits
I16 = mybir.dt.int16

D = 768          # d_model
DK = 64          # per-head dim
HPC = 3          # heads per core
P = 128          # SBUF partitions
QB = 512         # matmul moving-dim block
QH = QB // 2     # q-split half (256)
DT = D // P      # contraction k-tiles over d_model
N_CORES = 8
CH = 2           # kpos-tiles per pipeline chunk
AV_LAG = 6       # chunks between exp emission and AV consumption
EB_N = 24        # exp ring buffers

# Schraudolph constants: i16 = round(s_raw * (0.125*1024/ln2) + (15360-45)).
# Valid while |s/8| < 9.7 (measured range [-6.62, 6.32] on these inputs).
SCH_A = 128.0 / 0.6931471805599453
SCH_B = 15360.0 - 45.0
# chunk positions g (0..15) whose exp runs on DVE instead of ACT (6/16)
DVE_G = frozenset((1, 3, 6, 9, 11, 14))


def build_program(NQ=2048, NK=4096, repeat=1):
    KT_N = NK // P       # 32 kpos tiles
    QBS = NQ // QB       # 4 q blocks
    KB_N = NK // QB      # 8 kpos blocks for the K/V projection
    NCH = KT_N // CH     # 16 chunks per unit
    N_UNITS = HPC * QBS  # h-major: unit u = (h = u // QBS, qb = u % QBS)

    nc = bacc.Bacc("TRN2", target_bir_lowering=False, debug=False)

    xT = nc.dram_tensor("xT", [D, NQ], BF16, kind="ExternalInput").ap()
    encT = nc.dram_tensor("encT", [D, NK], BF16, kind="ExternalInput").ap()
    # wkv cols = [k0|v0 | v1|k1 | k2|v2]; projection pass pi=h uses cols
    # [128h, 128h+128)
    wkv = nc.dram_tensor("wkv", [D, 2 * HPC * DK], BF16, kind="ExternalInput").ap()
    wq = nc.dram_tensor("wq", [D, HPC * DK], BF16, kind="ExternalInput").ap()
    wo = nc.dram_tensor("wo", [HPC * DK, D], BF16, kind="ExternalInput").ap()
    bq = nc.dram_tensor("bq", [HPC * DK, 1], F32, kind="ExternalInput").ap()
    # bkv[:,0] = [bk_h0 ; bk_h1], bkv[:,1] = [bk_h2 ; 0]
    bkv = nc.dram_tensor("bkv", [P, 3], F32, kind="ExternalInput").ap()
    vfill = nc.dram_tensor("vfill", [1, 1, DK], BF16, kind="ExternalInput").ap()
    ones32d = nc.dram_tensor("ones32d", [1, 1, 32], BF16, kind="ExternalInput").ap()
    foldDd = nc.dram_tensor("foldDd", [P, 2], BF16, kind="ExternalInput").ap()
    rbSd = nc.dram_tensor("rbSd", [2, P], BF16, kind="ExternalInput").ap()
    yT = nc.dram_tensor("yT", [D, NQ], BF16, kind="ExternalOutput").ap()

    with tile.TileContext(nc) as tc, ExitStack() as ctx:
        consts = ctx.enter_context(tc.tile_pool(name="consts", bufs=1))
        persist = ctx.enter_context(tc.tile_pool(name="persist", bufs=1))
        stream = ctx.enter_context(tc.tile_pool(name="stream", bufs=2))
        small = ctx.enter_context(tc.tile_pool(name="small", bufs=2))
        o_pool = ctx.enter_context(tc.tile_pool(name="o_pool", bufs=1))
        ysb_pool = ctx.enter_context(tc.tile_pool(name="ysb", bufs=1))
        exp_pool = ctx.enter_context(tc.tile_pool(name="exp", bufs=EB_N))
        ps_s = ctx.enter_context(tc.tile_pool(name="ps_s", bufs=2, space="PSUM"))
        ps_av = ctx.enter_context(tc.tile_pool(name="ps_av", bufs=1, space="PSUM"))
        ps_den = ctx.enter_context(tc.tile_pool(name="ps_den", bufs=1, space="PSUM"))
        ps_o = ctx.enter_context(tc.tile_pool(name="ps_o", bufs=2, space="PSUM"))

        for _rep in range(repeat):
            # ---- constants (ordered by first use; wo deferred to fillers) --
            wq_sb = consts.tile([P, DT, HPC * DK], BF16)
            wkv_sb = consts.tile([P, DT, 2 * HPC * DK], BF16)
            wo_sb = consts.tile([DK, HPC, D], BF16)
            bqA = consts.tile([P, 1], F32)
            bqB = consts.tile([DK, 1], F32)
            bkv_sb = consts.tile([P, 3], F32)
            ones32 = consts.tile([P, 32], BF16)
            foldD = consts.tile([P, 2], BF16)
            rbS = consts.tile([2, P], BF16)

            def emit_early_consts():
                nc.sync.dma_start(out=wq_sb, in_=wq.rearrange("(t p) c -> p t c", p=P))

            def emit_mid_consts():
                nc.sync.dma_start(
                    out=wkv_sb, in_=wkv.rearrange("(t p) c -> p t c", p=P)
                )
                nc.sync.dma_start(out=bqA, in_=bq[0:P, :])
                nc.sync.dma_start(out=bqB, in_=bq[P : P + DK, :])
                nc.sync.dma_start(out=bkv_sb, in_=bkv)
                nc.sync.dma_start(
                    out=ones32.rearrange("p (a c) -> p a c", a=1),
                    in_=ones32d.to_broadcast([P, 1, 32]),
                )
                nc.sync.dma_start(out=foldD, in_=foldDd)
                nc.sync.dma_start(out=rbS, in_=rbSd)

            def emit_wo_const():
                nc.sync.dma_start(
                    out=wo_sb, in_=wo.rearrange("(h d) n -> d h n", d=DK)
                )

            # ---- persistent per-head tensors ------------------------------
            kT = [persist.tile([P, NK], BF16, name=f"kT{h}") for h in range(HPC)]
            qT = [persist.tile([P, NQ], BF16, name=f"qT{h}") for h in range(HPC)]
            v = [persist.tile([P, KT_N, 2 * DK], BF16, name=f"v{h}") for h in range(HPC)]
            vtA = persist.tile([P, NK], BF16, name="vtA")  # rows 64:128=V0, 0:64=V1
            vtB = persist.tile([P, NK], BF16, name="vtB")  # rows 64:128=V2

            # ---- pipeline state -------------------------------------------
            ebs = {}          # (u, g) -> exp tile
            u_ps = {}         # u -> (av psum tile, den psum tile)
            o_tiles = {}      # u -> normalized AV output [64, 512] bf16
            chunk_idx = {}    # (u, g) -> serial index when scores emitted
            chunk_count = 0
            av_count = 0
            av_work = [(u, g) for u in range(N_UNITS) for g in range(NCH)]
            v_emitted = [False] * HPC
            norm_q = []
            out_q = []
            fillers = []

            x_tiles = {}

            def x_dma(qb):
                qs = slice(qb * QB, (qb + 1) * QB)
                x_t = stream.tile([P, DT, QB], BF16, name="x_t", tag="x")
                nc.sync.dma_start(
                    out=x_t, in_=xT.rearrange("(t p) n -> p t n", p=P)[:, :, qs]
                )
                x_tiles[qb] = x_t

            enc_tiles = {}

            def enc_dma(b, phase):
                ks = slice(b * QB, (b + 1) * QB)
                enc_t = stream.tile([P, DT, QB], BF16, name="enc_t", tag="enc", bufs=3)
                nc.sync.dma_start(
                    out=enc_t, in_=encT.rearrange("(t p) n -> p t n", p=P)[:, :, ks]
                )
                enc_tiles[(b, phase)] = enc_t

            def emit_qproj(qb):
                qs = slice(qb * QB, (qb + 1) * QB)
                x_t = x_tiles.pop(qb)
                ps = ps_o.tile([P, QB], F32, tag="o", name="ps_q01")
                for t in range(DT):
                    nc.tensor.matmul(
                        ps, wq_sb[:, t, 0:P], x_t[:, t, :],
                        start=(t == 0), stop=(t == DT - 1),
                    )
                nc.vector.tensor_scalar_add(
                    out=qT[0][0:DK, qs], in0=ps[0:DK], scalar1=bqA[0:DK]
                )
                nc.vector.tensor_scalar_add(
                    out=qT[1][DK:P, qs], in0=ps[DK:P], scalar1=bqA[DK:P]
                )
                ps2 = ps_o.tile([P, QB], F32, tag="o", name="ps_q2")
                for t in range(DT):
                    nc.tensor.matmul(
                        ps2[0:DK], wq_sb[:, t, P : P + DK], x_t[:, t, :],
                        start=(t == 0), stop=(t == DT - 1),
                    )
                nc.vector.tensor_scalar_add(
                    out=qT[2][0:DK, qs], in0=ps2[0:DK], scalar1=bqB[0:DK]
                )
                nc.sync.dma_start(out=qT[0][DK:P, qs], in_=qT[0][0:DK, qs])
                nc.sync.dma_start(out=qT[1][0:DK, qs], in_=qT[1][DK:P, qs])
                nc.sync.dma_start(out=qT[2][DK:P, qs], in_=qT[2][0:DK, qs])

            def emit_kv_pi(b, pi, phase):
                """One projection pass (pi == head) on enc block b."""
                ks = slice(b * QB, (b + 1) * QB)
                enc_t = enc_tiles[(b, phase)]
                ps = ps_o.tile([P, QB], F32, tag="o", name="ps_kv")
                for t in range(DT):
                    nc.tensor.matmul(
                        ps, wkv_sb[:, t, pi * P : (pi + 1) * P],
                        enc_t[:, t, :], start=(t == 0), stop=(t == DT - 1),
                    )
                if pi == 0:
                    nc.vector.tensor_scalar_add(
                        out=kT[0][0:DK, ks], in0=ps[0:DK], scalar1=bkv_sb[0:DK, 0:1]
                    )
                    nc.sync.dma_start(out=kT[0][DK:P, ks], in_=kT[0][0:DK, ks])
                    nc.vector.tensor_copy(out=vtA[DK:P, ks], in_=ps[DK:P])
                elif pi == 1:
                    nc.vector.tensor_scalar_add(
                        out=kT[1][DK:P, ks], in0=ps[DK:P], scalar1=bkv_sb[DK:P, 0:1]
                    )
                    nc.sync.dma_start(out=kT[1][0:DK, ks], in_=kT[1][DK:P, ks])
                    nc.vector.tensor_copy(out=vtA[0:DK, ks], in_=ps[0:DK])
                else:
                    nc.vector.tensor_scalar_add(
                        out=kT[2][0:DK, ks], in0=ps[0:DK], scalar1=bkv_sb[0:DK, 1:2]
                    )
                    nc.sync.dma_start(out=kT[2][DK:P, ks], in_=kT[2][0:DK, ks])
                    nc.vector.tensor_copy(out=vtB[DK:P, ks], in_=ps[DK:P])

            def emit_v_transpose(h):
                src = {0: vtA[DK:P, :], 1: vtA[0:DK, :], 2: vtB[DK:P, :]}[h]
                nc.sync.dma_start(out=v[h][:, :, 0:DK], in_=src, transpose=True)
                v_emitted[h] = True

            def emit_scores_chunk(u, g):
                nonlocal chunk_count
                h, qb = u // QBS, u % QBS
                qs = slice(qb * QB, (qb + 1) * QB)
                ps = ps_s.tile([P, CH * QB], F32, tag="s", name="ps_sc")
                for j in range(CH):
                    kt = CH * g + j
                    half = slice(0, DK) if kt % 2 == 0 else slice(DK, P)
                    tp = (0, 0) if kt % 2 == 0 else (DK, 0)
                    nc.tensor.matmul(
                        ps[:, j * QB : (j + 1) * QB],
                        kT[h][half, kt * P : (kt + 1) * P],
                        qT[h][half, qs],
                        start=True, stop=True, tile_position=tp,
                    )
                eb = exp_pool.tile([P, CH * QB], BF16, tag="e", name="expT")
                if g in DVE_G:
                    # Schraudolph fast-exp on DVE: int16 bits are fp16 exp(s/8)
                    nc.vector.tensor_scalar(
                        out=eb.bitcast(I16), in0=ps,
                        scalar1=SCH_A, scalar2=SCH_B,
                        op0=mybir.AluOpType.mult, op1=mybir.AluOpType.add,
                    )
                else:
                    nc.scalar.activation(
                        out=eb, in_=ps,
                        func=mybir.ActivationFunctionType.Exp, scale=0.125,
                    )
                ebs[(u, g)] = eb
                chunk_idx[(u, g)] = chunk_count
                chunk_count += 1

            def emit_av_chunk(u, g):
                h = u // QBS
                if g == 0:
                    u_ps[u] = (
                        ps_av.tile([P, QB], F32, tag="av", name="ps_uav"),
                        ps_den.tile([P, QB], F32, tag="den", name="ps_uden"),
                    )
                ub, ud = u_ps[u]
                eb = ebs.pop((u, g))
                for j in range(CH):
                    kt = CH * g + j
                    ko = j * QB
                    # q-split col-tiled AV: A (q-lo) parts 0:64 pos 0,
                    # B (q-hi) parts 64:128 pos 64 -- concurrent pair
                    nc.tensor.matmul(
                        ub[0:DK, 0:QH], v[h][:, kt, 0:DK], eb[:, ko : ko + QH],
                        start=(kt == 0), stop=(kt == KT_N - 1),
                        tile_position=(0, 0),
                    )
                    nc.tensor.matmul(
                        ub[DK:P, 0:QH], v[h][:, kt, 0:DK], eb[:, ko + QH : ko + QB],
                        start=(kt == 0), stop=(kt == KT_N - 1),
                        tile_position=(0, DK),
                    )
                # 4-way col-tiled den pass: strips (even/odd kt) x (q-lo/q-hi)
                for qs0, tp in [(0, 0), (QH, 32), (QB, 64), (QB + QH, 96)]:
                    nc.tensor.matmul(
                        ud[tp : tp + 32, 0:QH],
                        ones32, eb[:, qs0 : qs0 + QH],
                        start=(g == 0), stop=(g == NCH - 1),
                        tile_position=(0, tp),
                    )

            def emit_unit_tail(u):
                ub, ud = u_ps.pop(u)
                denSB = small.tile([P, QH], BF16, tag="dsb", name="den_sb")
                nc.scalar.copy(out=denSB, in_=ud[:, 0:QH])
                dfold = ps_o.tile([2, QH], F32, tag="o", name="ps_dfold")
                nc.tensor.matmul(dfold, foldD, denSB, start=True, stop=True)
                rt = small.tile([2, QH], BF16, tag="rt", name="recip_t")
                with nc.allow_low_precision(reason="fp16 recip of softmax denom"):
                    nc.vector.reciprocal(out=rt, in_=dfold)
                rb = ps_o.tile([P, QH], F32, tag="o", name="ps_rb")
                nc.tensor.matmul(rb, rbS, rt, start=True, stop=True)
                rbsb = small.tile([P, QH], F32, tag="rbs", name="rb_sb")
                nc.scalar.copy(out=rbsb, in_=rb)
                o = small.tile([P, QH], BF16, tag="omul", name="o_mul")
                nc.vector.tensor_mul(out=o, in0=ub[:, 0:QH], in1=rbsb)
                # relocate q-halves into baseline o layout [64, 512]: mixing
                # row-positioned matmuls with the col-tiled AV/den stream
                # wedges the exec unit, so outproj stays all-(0,0)
                o2 = o_pool.tile([DK, QB], BF16, tag=f"o{u}", name="o_t")
                nc.sync.dma_start(out=o2[:, 0:QH], in_=o[0:DK, :])
                nc.sync.dma_start(out=o2[:, QH:QB], in_=o[DK:P, :])
                o_tiles[u] = o2

            ysb_tiles = {}

            def emit_outproj_step(qb, dt_i, on_act=False):
                qs = slice(qb * QB, (qb + 1) * QB)
                if dt_i == 0:
                    ysb_tiles[qb] = ysb_pool.tile([P, DT, QB], BF16, tag="y", name="y_t")
                ysb = ysb_tiles[qb]
                pso = ps_o.tile([P, QB], F32, tag="o", name="ps_o")
                ds = slice(dt_i * P, (dt_i + 1) * P)
                for h in range(HPC):
                    nc.tensor.matmul(
                        pso, wo_sb[:, h, ds], o_tiles[h * QBS + qb],
                        start=(h == 0), stop=(h == HPC - 1),
                    )
                if on_act:
                    nc.scalar.activation(
                        out=ysb[:, dt_i, :], in_=pso,
                        func=mybir.ActivationFunctionType.Copy,
                    )
                else:
                    nc.vector.tensor_copy(out=ysb[:, dt_i, :], in_=pso)
                nc.sync.dma_start(
                    out=yT.rearrange("(t p) n -> p t n", p=P)[:, dt_i, qs],
                    in_=ysb[:, dt_i, :],
                )
                if dt_i == DT - 1:
                    ysb_tiles.pop(qb)

            # ---- pipeline pumps -------------------------------------------
            def pump_av(budget=2, force=False):
                nonlocal av_count
                while budget > 0 and av_count < len(av_work):
                    u, g = av_work[av_count]
                    if not v_emitted[u // QBS]:
                        break
                    if (u, g) not in chunk_idx:
                        break
                    if not force and chunk_idx[(u, g)] > chunk_count - AV_LAG:
                        break
                    emit_av_chunk(u, g)
                    if g == NCH - 1:
                        norm_q.append(u)
                    av_count += 1
                    budget -= 1

            def pump_norm():
                while norm_q:
                    u = norm_q.pop(0)
                    emit_unit_tail(u)
                    if u // QBS == HPC - 1:
                        qb = u % QBS
                        for dt_i in range(DT):
                            out_q.append((qb, dt_i))

            def pump_out(budget=2, on_act=False):
                while budget > 0 and out_q:
                    qb, dt_i = out_q.pop(0)
                    emit_outproj_step(qb, dt_i, on_act=on_act)
                    budget -= 1

            def pump_filler():
                if fillers:
                    fillers.pop(0)()

            out_budget = 1
            filler_every = 1
            tick_n = 0

            def tick(u, g):
                nonlocal tick_n
                pump_av()
                emit_scores_chunk(u, g)
                pump_norm()
                pump_out(budget=out_budget)
                if tick_n % filler_every == 0:
                    pump_filler()
                tick_n += 1

            # ---- prologue: Q proj + K0/V0 proj + unit-0 scores ------------
            # Warmup matmuls on junk data first: bridge the x0/wq DMA wait so
            # the PE's HAM activity window is warm when real work lands.
            wscr = consts.tile([P, QB], BF16)
            nc.sync.dma_start(
                out=wscr.rearrange("p (a b) -> p a b", b=DK),
                in_=vfill.to_broadcast([P, QB // DK, DK]),
            )
            # DMA queue is FIFO: order strictly by first use.
            x_dma(0)
            emit_early_consts()
            enc_dma(0, 0)
            emit_mid_consts()
            enc_dma(1, 0)
            x_dma(1)
            for _w in range(8):
                ps_w = ps_o.tile([P, QB], F32, tag="o", name="ps_warm")
                nc.tensor.matmul(ps_w, wscr[:, 0:P], wscr, start=True, stop=True)
            emit_qproj(0)
            for b in range(KB_N):
                emit_kv_pi(b, 0, 0)
                if b == 0:
                    emit_qproj(1)
                tick(0, 2 * b)
                tick(0, 2 * b + 1)
                if b >= 2:
                    tick(1, 2 * (b - 2))
                    tick(1, 2 * (b - 2) + 1)
                if b + 2 < KB_N:
                    enc_dma(b + 2, 0)
            emit_v_transpose(0)

            # ---- steady-phase fillers -------------------------------------
            def mk(f, *a):
                return lambda: f(*a)

            fillers.extend([
                mk(x_dma, 2),
                mk(enc_dma, 0, 1),
                mk(emit_qproj, 2),
                mk(enc_dma, 1, 1),
                mk(x_dma, 3),
                mk(emit_qproj, 3),
                mk(emit_wo_const),
            ])
            for b in range(KB_N):
                fillers.append(mk(emit_kv_pi, b, 1, 1))
                fillers.append(mk(emit_kv_pi, b, 2, 1))
                if b + 2 < KB_N:
                    fillers.append(mk(enc_dma, b + 2, 1))
            fillers.append(mk(emit_v_transpose, 1))
            fillers.append(mk(emit_v_transpose, 2))

            # ---- steady phase ---------------------------------------------
            for u in range(1, N_UNITS):
                for g in range(NCH - 4 if u == 1 else 0, NCH):
                    tick(u, g)

            # ---- tail -----------------------------------------------------
            while av_count < len(av_work) or norm_q or out_q:
                pump_av(budget=4, force=True)
                pump_norm()
                pump_out(budget=6, on_act=True)
            while fillers:
                pump_filler()

    nc.compile()
    return nc


def shard_inputs(x, encoding, w_q, b_q, w_k, b_k, w_v, b_v, w_o, b_o):
    """Full inputs -> list of 8 per-core input dicts (numpy, contiguous)."""
    BF = np.float16
    N = x.shape[1]
    xT_full = np.asarray(x, np.float32)[0].T.astype(BF)      # [D, N]
    encT = np.ascontiguousarray(np.asarray(encoding, np.float32)[0].T.astype(BF))
    w_q, w_k, w_v, w_o = (np.asarray(a, np.float32) for a in (w_q, w_k, w_v, w_o))
    b_q, b_k = np.asarray(b_q, np.float32), np.asarray(b_k, np.float32)
    foldD = np.zeros((P, 2), np.float32)
    foldD[0:32, 0] = 1.0 / 32
    foldD[64:96, 0] = 1.0 / 32
    foldD[32:64, 1] = 1.0 / 32
    foldD[96:128, 1] = 1.0 / 32
    rbS = np.zeros((2, P), np.float32)
    rbS[0, 0:DK] = 1.0
    rbS[1, DK:P] = 1.0
    in_maps = []
    for core in range(N_CORES):
        p = core // 2
        hsel = slice(HPC * p * DK, HPC * (p + 1) * DK)
        qsel = slice(0, N // 2) if core % 2 == 0 else slice(N // 2, N)
        wk, wv = w_k[:, hsel], w_v[:, hsel]
        wkv = np.concatenate(
            [wk[:, 0:DK], wv[:, 0:DK],
             wv[:, DK:2*DK], wk[:, DK:2*DK],
             wk[:, 2*DK:3*DK], wv[:, 2*DK:3*DK]], axis=1
        ).astype(BF)
        bk = b_k[hsel]
        bkv = np.zeros((P, 3), np.float32)
        bkv[0:DK, 0] = bk[0:DK]
        bkv[DK:P, 0] = bk[DK:2*DK]
        bkv[0:DK, 1] = bk[2*DK:3*DK]
        in_maps.append(
            {
                "xT": np.ascontiguousarray(xT_full[:, qsel]),
                "encT": encT,
                "wkv": np.ascontiguousarray(wkv),
                "wq": np.ascontiguousarray(w_q[:, hsel].astype(BF)),
                "wo": np.ascontiguousarray(w_o[hsel, :].astype(BF)),
                "bq": np.ascontiguousarray(b_q[hsel].reshape(-1, 1)),
                "bkv": bkv,
                "vfill": np.concatenate(
                    [np.ones((1, 1, 1)), np.zeros((1, 1, DK - 1))], axis=2
                ).astype(BF),
                "ones32d": np.ones((1, 1, 32), BF),
                "foldDd": foldD.astype(BF),
                "rbSd": rbS.astype(BF),
            }
        )
    return in_maps


def combine_outputs(results, b_v, w_o, b_o, N, dtype):
    """Per-core yT partials -> full [1, N, D] output (host-side biases)."""
    half = N // 2
    y = np.zeros((N, D), np.float32)
    for core, res in enumerate(results):
        yT_part = np.asarray(res["yT"]).astype(np.float32)
        if core % 2 == 0:
            y[:half] += yT_part.T
        else:
            y[half:] += yT_part.T
    y += np.asarray(b_v, np.float32) @ np.asarray(w_o, np.float32) + np.asarray(
        b_o, np.float32
    )
    return np.ascontiguousarray(y[None]).astype(dtype)


_PROGRAM_CACHE = {}


def _get_program():
    key = "main"
    if key not in _PROGRAM_CACHE:
        _PROGRAM_CACHE[key] = build_program()
    return _PROGRAM_CACHE[key]


def kernel(x, encoding, w_q, b_q, w_k, b_k, w_v, b_v, w_o, b_o):
    nc = _get_program()
    in_maps = shard_inputs(x, encoding, w_q, b_q, w_k, b_k, w_v, b_v, w_o, b_o)
    res = run_bass_kernel_spmd(nc, in_maps, core_ids=list(range(N_CORES)))
    return combine_outputs(
        res.results, b_v, w_o, b_o, np.asarray(x).shape[1], np.asarray(x).dtype
    )
